# revision 11
# baseline (speedup 1.0000x reference)
"""AttentionAggregator Trainium2 kernel, v5.

Reference (per batch b, head h):
  qh = x_q @ Wq_h^T; kh = x @ Wk_h^T
  attn = softmax(qh @ kh^T / 8)
  heads_h = (attn @ r) @ Wv_h^T == attn @ (r @ Wv_h^T)   (associativity)
  out = concat_h(heads_h) @ Wo^T

Sharding: data-parallel over batch B=16 across 8 cores (2 batches/core).

Performance structure:
  - per-step serial chain scores(PE) -> exp(ACT, [128x1024] psum, ~1.15us)
    -> heads(PE); 64 steps per batch. All other matmuls (projections,
    transposes, output) are filler pieces emitted one per step, keeping
    the PE stream dense (HAM stays at 2.4 GHz).
  - all matmul operand paths bf16 (tolerance 2e-2; bf16 costs ~5e-3 rms).
  - input loads: ONE 2MB dma_start per tensor with row (8p+t) staged at
    stage[p, t] -> 16KB contiguous per-partition descriptors. A HWDGE
    ring generates ~20 packets/us, so bandwidth scales with descriptor
    size (2KB descs = ~40GB/s/ring, 16KB = ~320GB/s). The row
    permutation n = 8*nn + t is shared by q/k/v tensors and undone at
    the output store. Weights must stay in natural row order (permuting
    (h,e) rows would mix e-dims across heads) -> 2KB descriptors, but
    only 4MB total, on the other ring.
  - preamble split across both HWDGE rings (nc.sync + nc.scalar; the
    scalar ring is only safe before the exps start - it shares the ACT
    sequencer).
  - softmax normalize: denominators ride the heads matmul as a 65th
    stationary column of ones; 1/denom via DVE recip + GPSIMD
    partition_broadcast; the multiply reads the heads psum directly
    (no [65x512] evac copy). The whole chain runs before the next
    head's psum realloc; the resulting heads-matmul lag is absorbed by
    the attn tile pool depth.
"""

import sys

sys.path.insert(0, "/opt/trn_rl_repo")

import numpy as np

B, N, NQ, D, H = 16, 1024, 1024, 512, 8
HD = D // H  # 64
P = 128
NCORES = 8
BLOC = B // NCORES
ND = D // P    # 4 d-chunks
NM = N // P    # 8 m-tiles
NNQ = NQ // P  # 8 nq-tiles
FREE = 512

_CACHE = {}


def _build():
    import concourse.mybir as mybir
    from concourse.bacc import Bacc
    from concourse.tile import TileContext
    from concourse.masks import make_identity

    f32 = mybir.dt.float32
    bf16 = mybir.dt.bfloat16
    AF = mybir.ActivationFunctionType

    nc = Bacc("TRN2", target_bir_lowering=False, debug=False)

    x_d = nc.dram_tensor("x", [BLOC, N, D], f32, kind="ExternalInput")
    r_d = nc.dram_tensor("r", [BLOC, N, D], f32, kind="ExternalInput")
    xq_d = nc.dram_tensor("x_q", [BLOC, NQ, D], f32, kind="ExternalInput")
    wq_d = nc.dram_tensor("Wq", [H, HD, D], f32, kind="ExternalInput")
    wk_d = nc.dram_tensor("Wk", [H, HD, D], f32, kind="ExternalInput")
    wv_d = nc.dram_tensor("Wv", [H, HD, D], f32, kind="ExternalInput")
    wo_d = nc.dram_tensor("Wo", [D, D], f32, kind="ExternalInput")
    out_d = nc.dram_tensor("out", [BLOC, NQ, D], f32, kind="ExternalOutput")

    with TileContext(nc) as tc:
        with (
            tc.tile_pool(name="const", bufs=1) as constp,
            tc.tile_pool(name="wgt", bufs=1) as wgt,
            tc.tile_pool(name="big", bufs=1) as big,
            tc.tile_pool(name="stage", bufs=3) as stage,
            tc.tile_pool(name="attn", bufs=6) as attnp,
            tc.tile_pool(name="evac", bufs=4) as evacp,
            tc.tile_pool(name="ps1", bufs=2, space="PSUM") as ps1,
            tc.tile_pool(name="ps_sc", bufs=2, space="PSUM") as ps_sc,
            tc.tile_pool(name="ps_hd", bufs=2, space="PSUM") as ps_hd,
        ):
            ident = constp.tile([P, P], f32, name="ident")
            make_identity(nc, ident)
            ones_c = constp.tile([P, H, 2], bf16, name="ones_c")
            nc.any.memset(ones_c[:], 1.0)

            def warm():
                # transpose-mode ops don't count as PE-busy for the HAM
                # clock gate, so a preamble of pure transposes runs at
                # 1.2 GHz. A real matmul every few pieces keeps K=8/8.
                wp = ps1.tile([P, P], f32, tag="proj", name="warm")
                nc.tensor.matmul(wp[:], ident[:], ident[:], start=True,
                                 stop=True)

            # ---------- staged loads ----------
            def load_x(eng, dram_2d):
                """One 2MB input load, row 8p+t at st[p, t] (16KB descs)."""
                st = stage.tile([P, NM, D], f32, tag="stg2", name="stg2")
                eng.dma_start(
                    out=st[:], in_=dram_2d.rearrange("(p t) d -> p t d", t=NM))
                return st

            def load_w(eng, dram_rows):
                """1MB weight load, row 4p+t at st[p, t] (8KB descriptors;
                natural-order 2KB descriptors move at only ~35GB/s/ring).
                The (h,e) row permutation is undone by tp_wchunk's strided
                evac, so downstream layouts stay natural."""
                st = stage.tile([P, 4, D], f32, tag="stgw", name="stgw",
                                bufs=3)
                eng.dma_start(
                    out=st[:], in_=dram_rows.rearrange("(p t) d -> p t d", t=4))
                return st

            def tp_wchunk(st, dst, k):
                """Transpose d-chunk k of a permuted-staged weight and
                unpermute on evac: psum col nn holds weight row 4nn+t, which
                is (he) = 128s + 4n + t at dst slot s, col 4n+t."""
                pt = ps1.tile([P, FREE], f32, tag="proj", name="tpw")
                for t in range(4):
                    nc.tensor.transpose(
                        pt[:, t * P:(t + 1) * P],
                        st[:, t, k * P:(k + 1) * P], ident[:])
                dv = dst[:, k, :, :].rearrange("p s (n e) -> p s n e", e=4)
                for t in range(4):
                    nc.vector.tensor_copy(
                        dv[:, :, :, t], pt[:, t * P:(t + 1) * P])

            def tp_piece(st, dst, k, h2):
                """Transpose 4 staged slots (t = 4*h2..) of d-chunk k into
                dst[:, k, 4*h2:4*h2+4, :] (one psum tile + one evac)."""
                pt = ps1.tile([P, FREE], f32, tag="proj", name="tp")
                for i in range(4):
                    nc.tensor.transpose(
                        pt[:, i * P:(i + 1) * P],
                        st[:, 4 * h2 + i, k * P:(k + 1) * P], ident[:])
                nc.vector.tensor_copy(dst[:, k, 4 * h2:4 * (h2 + 1), :], pt[:])

            def alloc_T(nm, b):
                # tags shared across batches: batch 1's transposes land
                # after batch 0's last projection read of the same tensor
                return big.tile([P, ND, NM, P], bf16, tag=nm, name=f"{nm}_{b}")

            # ---------- projections (emittable in pieces) ----------
            def alloc_proj(b):
                qhT = [big.tile([P, NQ], bf16, tag=f"qhT_{hp}",
                                name=f"qhT_{hp}_{b}") for hp in range(4)]
                khT = [big.tile([P, N], bf16, tag=f"khT_{hp}",
                                name=f"khT_{hp}_{b}") for hp in range(4)]
                vh = [big.tile([P, H, 66], bf16, tag=f"vh_{m}_{b}",
                               name=f"vh_{m}_{b}") for m in range(NM)]
                return qhT, khT, vh

            def proj_one(wT, xt, dst, hp, c):
                pp = ps1.tile([P, FREE], f32, tag="proj", name="proj")
                for k in range(ND):
                    nc.tensor.matmul(
                        pp[:], wT[:, k, hp, :],
                        xt[:, k, 4 * c:4 * (c + 1), :],
                        start=(k == 0), stop=(k == ND - 1))
                nc.vector.tensor_copy(dst[hp][:, c * FREE:(c + 1) * FREE], pp[:])

            def proj_vh(rt, vh, m):
                pp = ps1.tile([P, FREE], f32, tag="proj", name="proj")
                for k in range(ND):
                    nc.tensor.matmul(
                        pp[:], rt[:, k, m, :],
                        wvT[:, k, :, :].rearrange("p a b -> p (a b)"),
                        start=(k == 0), stop=(k == ND - 1))
                nc.vector.tensor_copy(
                    vh[m][:, :, 0:HD], pp[:].rearrange("p (h e) -> p h e", h=H))
                nc.vector.tensor_copy(vh[m][:, :, 64:66], ones_c[:])

            def qk_pieces(tin, qhT, khT, hp):
                return [
                    lambda c=c, w=w, x=x, d=d: proj_one(w, tin[x], d, hp, c)
                    for c in range(2) for w, x, d in
                    ((wqT, "xqT", qhT), (wkT, "xT", khT))]

            def out_tile(b, concatT, t):
                po = ps1.tile([P, D], f32, tag="proj", name="proj")
                for hp in range(4):
                    nc.tensor.matmul(
                        po[:], concatT[hp][:, t * P:(t + 1) * P],
                        woT[:, hp, :, :].rearrange("p a b -> p (a b)"),
                        start=(hp == 0), stop=(hp == 3))
                ot = evacp.tile([P, D], f32, tag="out", name="out", bufs=2)
                nc.vector.tensor_copy(ot[:], po[:])
                # undo the staging row permutation: concatT col t*128+nn is
                # query n = 8*nn + t
                dst = out_d.ap()[b].rearrange("(p t) d -> t p d", t=NM)
                nc.sync.dma_start(out=dst[t], in_=ot[:])

            # ---------- attention ----------
            def attention(b, qhT, khT, vh, queue):
                concatT = [big.tile([P, NQ], bf16, tag=f"concatT_{hp}_{b}",
                                    name=f"concatT_{hp}_{b}") for hp in range(4)]
                n_steps = H * NM
                pending = {}

                def norm_evac(h, c):
                    """Evacuate heads psum c-half to SBUF (frees ph).
                    c=0 is emitted between the two m=7 heads matmuls so
                    only c=1's evac sits before the next ph realloc."""
                    st = pending[h]
                    if "hc" not in st:
                        st["hc"] = [None, None]
                    hc = evacp.tile([65, FREE], f32, tag="hcopy", name="hcopy")
                    nc.vector.tensor_copy(hc[:], st["ph"][c][:])
                    st["hc"][c] = hc

                def norm_recip(h):
                    st = pending[h]
                    st["rec"] = []
                    for c in range(2):
                        # recip_approx misreads inputs not based at
                        # partition 0 — stage through a partition-0 tile
                        dcp = evacp.tile([1, FREE], f32, tag="dcp", name="dcp",
                                         bufs=2)
                        nc.vector.tensor_copy(dcp[:], st["hc"][c][64:65, :])
                        rec = evacp.tile([1, FREE], f32, tag="rec", name="rec",
                                         bufs=2)
                        nc.vector.reciprocal_approx_fast(rec[:], dcp[:])
                        st["rec"].append(rec)

                def norm_bcast(h):
                    st = pending[h]
                    st["bcp"] = []
                    for c in range(2):
                        bcp = evacp.tile([HD, FREE], f32, tag="bcp",
                                         name="bcp", bufs=2)
                        nc.gpsimd.partition_broadcast(bcp[:], st["rec"][c][:])
                        st["bcp"].append(bcp)

                def norm_mul(h):
                    st = pending.pop(h)
                    hp, off = h // 2, (h % 2) * HD
                    for c in range(2):
                        nc.vector.tensor_mul(
                            concatT[hp][off:off + HD, c * FREE:(c + 1) * FREE],
                            st["hc"][c][0:HD, :], st["bcp"][c][:])

                def score_mm(j):
                    h, m = divmod(j, NM)
                    hp, off = h // 2, (h % 2) * HD
                    psc = ps_sc.tile([P, NQ], f32, tag="score", name="score")
                    for c in range(NQ // FREE):
                        nc.tensor.matmul(
                            psc[:, c * FREE:(c + 1) * FREE],
                            khT[hp][off:off + HD, m * P:(m + 1) * P],
                            qhT[hp][off:off + HD, c * FREE:(c + 1) * FREE],
                            start=True, stop=True)
                    return psc

                ph = None
                psc_cur = score_mm(0)
                for j in range(n_steps):
                    h, m = divmod(j, NM)
                    if m == 0:
                        if h > 0:
                            norm_evac(h - 1, 1)
                        ph = [ps_hd.tile([65, FREE], f32, tag="heads",
                                         name=f"heads{c}") for c in range(2)]
                    at = attnp.tile([P, NQ], bf16, tag="attnT", name="attnT")
                    nc.scalar.activation(at[:], psc_cur[:], AF.Exp, scale=0.125)
                    if j + 1 < n_steps:
                        psc_cur = score_mm(j + 1)
                    if queue:
                        th = queue.pop(0)
                        if th is not None:
                            th()
                    for c in range(2):
                        nc.tensor.matmul(
                            ph[c][:], vh[m][:, h, 0:65],
                            at[:, c * FREE:(c + 1) * FREE],
                            start=(m == 0), stop=(m == NM - 1))
                        if m == NM - 1 and c == 0:
                            pending[h] = {"ph": ph}
                            norm_evac(h, 0)
                    if h > 0:
                        if m == 2:
                            norm_recip(h - 1)
                        elif m == 4:
                            norm_bcast(h - 1)
                        elif m == 6:
                            norm_mul(h - 1)
                norm_evac(H - 1, 1)
                norm_recip(H - 1)
                norm_bcast(H - 1)
                norm_mul(H - 1)
                return concatT

            # ---------- schedule ----------
            wqT = wgt.tile([P, ND, 4, P], bf16, tag="wqT", name="wqT")
            wkT = wgt.tile([P, ND, 4, P], bf16, tag="wkT", name="wkT")
            wvT = wgt.tile([P, ND, 4, P], bf16, tag="wvT", name="wvT")
            woT = wgt.tile([P, ND, 4, P], bf16, tag="woT", name="woT")

            tin0 = {"xqT": alloc_T("xqT", 0), "xT": alloc_T("xT", 0),
                    "rT": alloc_T("rT", 0)}
            tin1 = {"xqT": alloc_T("xqT", 1), "xT": alloc_T("xT", 1),
                    "rT": alloc_T("rT", 1)}
            q0, k0, v0 = alloc_proj(0)
            q1, k1, v1 = alloc_proj(1)

            wq_rows = wq_d.ap().rearrange("h e d -> (h e) d")
            wk_rows = wk_d.ap().rearrange("h e d -> (h e) d")
            wv_rows = wv_d.ap().rearrange("h e d -> (h e) d")

            # preamble: ring A (sync): xq, x, r (2MB each, 16KB descs);
            # ring B (scalar): wq, wk, wv (1MB each, 2KB descs).
            # exp0 needs wq/wk + xq/x; vh m0-3 need wv + r (first half of
            # the transposes); r's second half + vh m4-7 spill into attn0.
            # preamble loads over three DMA paths:
            #   ring A (sync):    xq (16KB descs), wv (8KB)
            #   ring B (scalar):  wq, x   — safe pre-attention only
            #   ring G (gpsimd):  wk, r
            st_wq = load_w(nc.sync, wq_rows)
            st_wk = load_w(nc.gpsimd, wk_rows)
            st_xq = load_x(nc.sync, xq_d.ap()[0])
            st_x = load_x(nc.sync, x_d.ap()[0])
            st_r = load_x(nc.sync, r_d.ap()[0])
            st_wv = load_w(nc.scalar, wv_rows)
            for k in range(ND):
                tp_wchunk(st_wq, wqT, k)
                warm()
            for k in range(ND):
                tp_wchunk(st_wk, wkT, k)
                warm()
            for h2 in range(2):
                for k in range(ND):
                    tp_piece(st_xq, tin0["xqT"], k, h2)
                    warm()
            for h2 in range(2):
                for k in range(ND):
                    tp_piece(st_x, tin0["xT"], k, h2)
                    warm()
            proj_one(wqT, tin0["xqT"], q0, 0, 0)
            proj_one(wkT, tin0["xT"], k0, 0, 0)
            proj_one(wqT, tin0["xqT"], q0, 0, 1)
            proj_one(wkT, tin0["xT"], k0, 0, 1)
            for k in range(ND):
                tp_wchunk(st_wv, wvT, k)
                warm()
            for k in range(ND):
                tp_piece(st_r, tin0["rT"], k, 0)
                warm()
            for m in range(4):
                proj_vh(tin0["rT"], v0, m)

            # batch-1 inputs: load (ring A) -> one whole-tensor DVE cast
            # -> 8 xbar transposes (ring A). No PE cost.
            st1 = {}
            stb1 = {}

            def b1_load(nm):
                src = {"xqT": xq_d, "xT": x_d, "rT": r_d}[nm]
                st1[nm] = load_x(nc.sync, src.ap()[1])

            def b1_cast(nm, g):
                natb = stage.tile([P, 4, D], bf16, tag="stgb", name="stgb",
                                  bufs=2)
                nc.vector.tensor_copy(natb[:], st1[nm][:, 4 * g:4 * (g + 1), :])
                stb1[(nm, g)] = natb

            def b1_xbar(nm, g):
                for t in range(4):
                    nc.sync.dma_start_transpose(
                        tin1[nm][:, :, 4 * g + t, :], stb1[(nm, g)][:, t, :])

            wo_st = {}

            def wo_load():
                wo_st["st"] = load_w(nc.sync, wo_d.ap())

            # queue0: piece p runs ~step p of attn0 (64 steps). Deadlines:
            # b0 vh[m] step m (m4-7 -> pieces 4-7); b0 qk hp from step
            # 16*hp; b1 transposes overwrite tag-shared tin tiles after
            # b0's last projection read (piece 19); b1 qk hp0 after b0's
            # last hp0 score read (step 15).
            queue0 = [lambda k=k: tp_piece(st_r, tin0["rT"], k, 1)
                      for k in range(ND)]                           # 0-3
            queue0[0] = (lambda th=queue0[0]: (b1_load("xqT"), th()))
            queue0[1] = (lambda th=queue0[1]: (b1_load("xT"), th()))
            queue0[2] = (lambda th=queue0[2]: (b1_load("rT"), th()))
            queue0 += [lambda m=m: proj_vh(tin0["rT"], v0, m)
                       for m in range(4, NM)]                       # 4-7
            pcs = qk_pieces(tin0, q0, k0, 1)                        # 8-11
            queue0[4] = (lambda th=queue0[4]: (b1_cast("xqT", 0), th()))
            queue0[5] = (lambda th=queue0[5]: (b1_cast("xqT", 1), th()))
            queue0 += [lambda th=pcs[0]: (b1_xbar("xqT", 0), th()),
                       lambda th=pcs[1]: (b1_cast("xT", 0), th()),
                       lambda th=pcs[2]: (b1_xbar("xqT", 1), th()),
                       lambda th=pcs[3]: (b1_cast("xT", 1), th())]
            pcs = qk_pieces(tin0, q0, k0, 2)                        # 12-15
            queue0 += [lambda th=pcs[0]: (b1_xbar("xT", 0), th()),
                       lambda th=pcs[1]: (b1_cast("rT", 0), th()),
                       lambda th=pcs[2]: (b1_xbar("xT", 1), th()),
                       lambda th=pcs[3]: (b1_cast("rT", 1), th())]
            pcs = qk_pieces(tin0, q0, k0, 3)                        # 16-19
            queue0 += [lambda th=pcs[0]: (b1_xbar("rT", 0), th()),
                       lambda th=pcs[1]: (b1_xbar("rT", 1), th())] + pcs[2:]
            queue0 += [None] * 11                                   # 20-30
            queue0 += qk_pieces(tin1, q1, k1, 0)                    # 31-34
            queue0 += [lambda m=m: proj_vh(tin1["rT"], v1, m)
                       for m in range(4)]                           # 35-38
            queue0.append(wo_load)                                  # 39
            queue0 += [lambda k=k: tp_wchunk(wo_st["st"], woT, k)
                       for k in range(ND)]                          # 40-43

            c0 = attention(0, q0, k0, v0, queue0)

            # queue1: b1 vh m4-7 (deadlines steps 4-7), b1 qk hp1-3
            # (deadlines 16/32/48), b0 out tiles
            queue1 = [lambda m=m: proj_vh(tin1["rT"], v1, m)
                      for m in range(4, NM)]
            for hp in range(1, 4):
                queue1 += qk_pieces(tin1, q1, k1, hp)
            queue1 += [lambda t=t: out_tile(0, c0, t) for t in range(NNQ)]

            c1 = attention(1, q1, k1, v1, queue1)
            for t in range(NNQ):
                out_tile(1, c1, t)

    nc.finalize()
    return nc


def _get_nc():
    if "nc" not in _CACHE:
        _CACHE["nc"] = _build()
    return _CACHE["nc"]


def kernel(x, r, x_q, Wq, Wk, Wv, Wo, **kw):
    from concourse.bass_utils import run_bass_kernel_spmd

    nc = _get_nc()
    x = np.ascontiguousarray(x, np.float32)
    r = np.ascontiguousarray(r, np.float32)
    x_q = np.ascontiguousarray(x_q, np.float32)
    in_maps = []
    for c in range(NCORES):
        sl = slice(c * BLOC, (c + 1) * BLOC)
        in_maps.append({
            "x": x[sl], "r": r[sl], "x_q": x_q[sl],
            "Wq": np.ascontiguousarray(Wq, np.float32),
            "Wk": np.ascontiguousarray(Wk, np.float32),
            "Wv": np.ascontiguousarray(Wv, np.float32),
            "Wo": np.ascontiguousarray(Wo, np.float32),
        })
    res = run_bass_kernel_spmd(nc, in_maps, list(range(NCORES)), **kw)
    out = np.concatenate([res.results[c]["out"] for c in range(NCORES)], axis=0)
    _CACHE["last_results"] = res
    return out
